# revision 46
# baseline (speedup 1.0000x reference)
"""Trainium2 Bass kernel for the ActorNetwork GNN problem (self-contained).

Strategy
--------
The batched graph is identical for every batch element (the reference's
"offset trick"), so the normalized adjacency P = D^-1/2 (A+I) D^-1/2
[5000 x 5000] is shared across all 16 batch elements and both GCN layers.
Per-edge gather/scatter is hostile to Trainium (descriptor-rate bound), so
the aggregation is done as a *dense* matmul with P sharded by destination
node across the 8 cores: each core holds a [5120 x 640] fp8 slice of P^T
(SBUF-resident, built on the host from edge_index) and aggregates for all
16 batch elements at once (fp8 DoubleRow).

Measured collective facts on this fabric: each collective op costs
~12us fixed + ~4us/MB, so the hidden features H [5120, 256] fp8 are
exchanged with ONE full-width AllGather per layer (group-split AGs were
measured strictly worse). The collective stack init barrier starts at
CC-core boot (~21us, independent of any doorbell) and takes 28-53us;
the first op after it pays a further ~11us setup. A warmup dummy
collective only ADDS its own ~10us — omitted. AG payloads are
partition-major per slab so every DMA moves >=512B-contiguous
per-partition elements (128B-strided DMAs run ~20GB/s vs ~115GB/s).

The Tensor clock drops to ~1.2GHz after a few us of idle and needs
several us of HIGH-DUTY work to return to 2.4GHz (128-wide ops at ~17%
stream duty barely ramp it). Six 512-wide fp8-DR scratch matmuls, gated
on a small probe DMA of the gathered output so they start exactly at
data arrival, re-warm the clock after each AllGather wait; large static
filler fleets are both untimeable (barrier variance) and self-defeating
(their heat inflates the next collective by up to 2x).
Per-group (8-batch) interleaving runs group 0's
relu/mm2/head epilogues underneath group 1's aggregation chains; the
aggregation is split into two accumulation chains (cols 0:512,
512:640) so each group's epilogue starts before its chain fully ends.
The joint head: even node tiles via narrow fc matmul + DVE
broadcast-add (st2 = nl[dst,b] + col_logits[b,c]), odd tiles via the
512-wide fcbc matmul on the (tail-idle) Tensor engine.

Everything is node-sharded: core c owns true nodes [c*625, (c+1)*625),
padded to 640 (= 5 x 128). Global padded node id = c*640 + local.
"""

import numpy as np
import ml_dtypes

BF16NP = ml_dtypes.bfloat16
FP8NP = ml_dtypes.float8_e4m3

B, N, F, E, C, FC = 16, 5000, 512, 160000, 64, 128
NCORES = 8
NLOC = N // NCORES            # 625 true nodes per core
NPAD = 640                    # padded nodes per core (5 x 128)
NT = NPAD // 128              # node tiles per core
NG = NCORES * NPAD            # 5120 padded global nodes
KT = NG // 128                # 40 src k-tiles
HF = 16                       # hidden feature width
GB = 8                        # batch elements per partition group
NGRP = B // GB                # 2 groups
GW = GB * HF                  # 128 = per-group (b,f) width
BFW = B * HF                  # 256 = full (batch, feat) width
FKT = F // 128                # 4 k-tiles for the input features
N_FILL1 = 0                 # p-state warmup matmuls before layer-1 agg
N_FILL2 = 0                # p-state warmup matmuls before layer-2 agg
DUMMY = False                 # zero-dep warmup collective at preamble end

_GRAPH_CACHE = {}


# --------------------------------------------------------------------------
# Host-side preprocessing (index/layout work only)
# --------------------------------------------------------------------------

def _preprocess(inputs):
    nf = np.asarray(inputs["node_features"], dtype=np.float32)   # [B, N, F]
    cf = np.asarray(inputs["col_features"], dtype=np.float32)    # [B, C, FC]
    ei = np.asarray(inputs["edge_index"])                        # [2, E] int64

    src = ei[0].astype(np.int64)
    dst = ei[1].astype(np.int64)

    # Degrees / normalization exactly as the reference (in-degree + self loop)
    deg = np.bincount(dst, minlength=N).astype(np.float64) + 1.0
    dinv = 1.0 / np.sqrt(deg)
    norm = (dinv[src] * dinv[dst]).astype(np.float32)

    # Dense P^T [src_padded_global, dst_padded_global], f32 accumulate
    pg = lambda n: (n // NLOC) * NPAD + (n % NLOC)
    PT = np.zeros((NG, NG), dtype=np.float32)
    np.add.at(PT, (pg(src), pg(dst)), norm)
    loop = np.arange(N, dtype=np.int64)
    pl = pg(loop)
    PT[pl, pl] += (dinv * dinv).astype(np.float32)

    # P^T slices, p-major for contiguous DMA: [128, KT*NPAD] fp8.
    pt_cores = [
        np.ascontiguousarray(
            PT[:, c * NPAD:(c + 1) * NPAD].astype(FP8NP)
            .reshape(KT, 128, NPAD).transpose(1, 0, 2)
            .reshape(128, KT * NPAD))
        for c in range(NCORES)
    ]

    # X^T slices, per-group contiguous: [NGRP, 128, GB*FKT*NPAD] where
    # row p, slot j = b_local*FKT + k holds X^T[k*128+p, :] of batch
    # g*GB+b_local. DMA'd per batch (16 DMAs across two queues).
    xt_cores = []
    for c in range(NCORES):
        xt = np.zeros((B, F, NPAD), dtype=FP8NP)
        xt[:, :, :NLOC] = nf[:, c * NLOC:(c + 1) * NLOC, :].transpose(0, 2, 1)
        xt = np.ascontiguousarray(
            xt.reshape(NGRP, GB, FKT, 128, NPAD).transpose(0, 3, 1, 2, 4)
            .reshape(NGRP, 128, GB * FKT * NPAD))
        xt_cores.append(xt)

    # Block-diagonal W1 for the grouped mm1: out[(b,f), n] = sum over the
    # (b', F) contraction of blockdiag(W1). Tile j = b'*FKT + k holds
    # W1[k*128+p, f] in columns b'*HF..(b'+1)*HF.
    W1f = np.asarray(inputs["W1"], np.float32)
    wblk1 = np.zeros((128, GB * FKT, 128), dtype=FP8NP)
    for b_ in range(GB):
        for k in range(FKT):
            wblk1[:, b_ * FKT + k, b_ * HF:(b_ + 1) * HF] = \
                W1f[k * 128:(k + 1) * 128, :].astype(FP8NP)

    # Column features transposed: [FC, B*C] bf16 (replicated)
    cft = np.ascontiguousarray(
        cf.transpose(2, 0, 1).reshape(FC, B * C)).astype(BF16NP)

    W2 = np.asarray(inputs["W2"], np.float32)
    fc_w = np.asarray(inputs["fc_w"], np.float32)
    fc_b = np.asarray(inputs["fc_b"], np.float32)
    cw1 = np.asarray(inputs["cw1"], np.float32)
    cb1 = np.asarray(inputs["cb1"], np.float32)
    cw2 = np.asarray(inputs["cw2"], np.float32)
    cb2 = np.asarray(inputs["cb2"], np.float32)
    b1 = np.asarray(inputs["b1"], np.float32)
    b2 = np.asarray(inputs["b2"], np.float32)

    # narrow fc head: col (b') of block b holds fc_w iff b'==b
    fcblk = np.kron(np.eye(GB, dtype=np.float32), fc_w).astype(BF16NP)
    # wide fc head (for the Tensor-engine path on odd node tiles)
    fcbc = np.kron(np.eye(GB, dtype=np.float32),
                   fc_w @ np.ones((1, C), dtype=np.float32)).astype(BF16NP)

    shared = {
        "cft": cft,
        "wblk1": wblk1.reshape(128, GB * FKT * 128),
        "wblk": np.kron(np.eye(GB, dtype=np.float32), W2).astype(BF16NP),
        "fcblk": fcblk,
        "fcbc": fcbc,
        "cw1": cw1.astype(BF16NP),
        "cw2": cw2.astype(BF16NP),
        "b1t": np.tile(b1, GB)[:, None].astype(np.float32),
        "b2t": np.tile(b2, GB)[:, None].astype(np.float32),
        "cb1": cb1[:, None].astype(np.float32),
        "clb": np.array([[fc_b[0] + cb2[0]]], dtype=np.float32),
    }
    return xt_cores, pt_cores, shared


# --------------------------------------------------------------------------
# Device graph (identical on all 8 cores)
# --------------------------------------------------------------------------

def _build_graph():
    from concourse import bacc
    import concourse.mybir as mybir
    import concourse.tile as tile
    from concourse.bass import ts, broadcast_tensor_aps
    from concourse.masks import make_identity

    f32 = mybir.dt.float32
    bf16 = mybir.dt.bfloat16
    fp8 = mybir.dt.float8e4
    AF = mybir.ActivationFunctionType
    DR = mybir.MatmulPerfMode.DoubleRow
    KT2 = KT // 2         # 20 paired src k-tiles

    nc = bacc.Bacc("TRN2", target_bir_lowering=False, debug=False,
                   num_devices=NCORES)

    xt_e = nc.dram_tensor("xt", [NGRP, 128, GB * FKT * NPAD], fp8,
                          kind="ExternalInput")
    pt_e = nc.dram_tensor("pt", [128, KT * NPAD], fp8, kind="ExternalInput")
    cft_e = nc.dram_tensor("cft", [FC, B * C], bf16, kind="ExternalInput")
    wblk1_e = nc.dram_tensor("wblk1", [128, GB * FKT * 128], fp8,
                             kind="ExternalInput")
    wblk_e = nc.dram_tensor("wblk", [128, 128], bf16, kind="ExternalInput")
    fcblk_e = nc.dram_tensor("fcblk", [128, GB], bf16, kind="ExternalInput")
    fcbc_e = nc.dram_tensor("fcbc", [128, GB * C], bf16,
                            kind="ExternalInput")
    cw1_e = nc.dram_tensor("cw1", [FC, HF], bf16, kind="ExternalInput")
    cw2_e = nc.dram_tensor("cw2", [HF, 1], bf16, kind="ExternalInput")
    b1_e = nc.dram_tensor("b1t", [128, 1], f32, kind="ExternalInput")
    b2_e = nc.dram_tensor("b2t", [128, 1], f32, kind="ExternalInput")
    cb1_e = nc.dram_tensor("cb1", [HF, 1], f32, kind="ExternalInput")
    clb_e = nc.dram_tensor("clb", [1, 1], f32, kind="ExternalInput")
    out_e = nc.dram_tensor("out", [NGRP, 128, NT * GB * C], bf16,
                           kind="ExternalOutput")

    rg = [list(range(NCORES))]

    with tile.TileContext(nc) as tc:
        with (
            tc.tile_pool(name="const", bufs=1) as constp,
            tc.tile_pool(name="ptp", bufs=1) as ptp,
            tc.tile_pool(name="hallp", bufs=1) as hallp,
            tc.tile_pool(name="rowsp", bufs=1) as rowsp,
            tc.tile_pool(name="xlp", bufs=1) as xlp,
            tc.tile_pool(name="xsp", bufs=1) as xsp,
            tc.tile_pool(name="stg2p", bufs=1) as stg2p,
            tc.tile_pool(name="dram", bufs=1, space="DRAM") as dramp,
            tc.tile_pool(name="ps", bufs=2, space="PSUM") as ps,
        ):
            # ---- dummy warmup AllGather: zero-dependency doorbell.
            if DUMMY:
                dum_in = dramp.tile([1, 64], fp8, name="dum_in")
                dum_out = dramp.tile([1, 64], fp8, addr_space="Shared",
                                     name="dum_out")
                nc.gpsimd.collective_compute(
                    "AllGather",
                    mybir.AluOpType.bypass,
                    replica_groups=[[c] for c in range(NCORES)],
                    ins=[dum_in[:].opt()],
                    outs=[dum_out[:].opt()],
                )

            # ---- critical-path DMAs first: W1 blockdiag, then X per batch
            # alternating the two DMA-capable queues.
            wblk1_sb = constp.tile([128, GB * FKT, 128], fp8, name="wblk1_sb")
            nc.sync.dma_start(
                out=wblk1_sb[:].rearrange("p k f -> p (k f)"), in_=wblk1_e[:])
            ident_sb = constp.tile([128, 128], bf16, name="ident_sb")
            make_identity(nc, ident_sb[:])
            xg_tiles = []
            BW = FKT * NPAD       # per-batch column width in xt
            for g in range(NGRP):
                xg = xsp.tile([128, GB * FKT, NPAD], fp8, tag=f"xg{g}",
                              name=f"xg_{g}")
                for j in range(GB):
                    eng = nc.sync if j % 2 == 0 else nc.scalar
                    eng.dma_start(
                        out=xg[:, j * FKT:(j + 1) * FKT, :]
                        .rearrange("p k n -> p (k n)"),
                        in_=xt_e[g][:, j * BW:(j + 1) * BW])
                xg_tiles.append(xg)
            pt_sb = ptp.tile([128, KT, NPAD], fp8, name="pt_sb")

            # ---- remaining constants
            wblk_sb = constp.tile([128, 128], bf16, name="wblk_sb")
            nc.sync.dma_start(out=wblk_sb[:], in_=wblk_e[:])
            fcblk_sb = constp.tile([128, GB], bf16, name="fcblk_sb")
            nc.sync.dma_start(out=fcblk_sb[:], in_=fcblk_e[:])
            fcbc_sb = constp.tile([128, GB * C], bf16, name="fcbc_sb")
            nc.sync.dma_start(out=fcbc_sb[:], in_=fcbc_e[:])
            cw1_sb = constp.tile([FC, HF], bf16, name="cw1_sb")
            nc.sync.dma_start(out=cw1_sb[:], in_=cw1_e[:])
            cw2_sb = constp.tile([HF, 1], bf16, name="cw2_sb")
            nc.sync.dma_start(out=cw2_sb[:], in_=cw2_e[:])
            b1_sb = constp.tile([128, 1], f32, name="b1_sb")
            nc.sync.dma_start(out=b1_sb[:], in_=b1_e[:])
            b2_sb = constp.tile([128, 1], f32, name="b2_sb")
            nc.sync.dma_start(out=b2_sb[:], in_=b2_e[:])
            cb1_sb = constp.tile([HF, 1], f32, name="cb1_sb")
            nc.sync.dma_start(out=cb1_sb[:], in_=cb1_e[:])
            clb_sb = constp.tile([1, 1], f32, name="clb_sb")
            nc.sync.dma_start(out=clb_sb[:], in_=clb_e[:])
            cft_sb = constp.tile([FC, B * C], bf16, name="cft_sb")
            nc.sync.dma_start(out=cft_sb[:], in_=cft_e[:])
            ones_sb = constp.tile([1, 128], bf16, name="ones_sb")
            nc.vector.memset(ones_sb[:], 1.0)

            # AllGather payloads are partition-major per slab
            # ([128, NT*BFW] per core) so both the pre-AG writes and the
            # post-AG SBUF loads move contiguous >=1280B elements per
            # partition (128B-strided loads measured ~20GB/s; contiguous
            # ~115GB/s per queue).
            def ag_issue(layer, ag_in):
                ag_out = dramp.tile([NCORES, 128, NT * BFW], fp8,
                                    addr_space="Shared", name=f"ago{layer}")
                nc.gpsimd.collective_compute(
                    "AllGather",
                    mybir.AluOpType.bypass,
                    replica_groups=rg,
                    ins=[ag_in[:].opt()],
                    outs=[ag_out[:].opt()],
                )
                return ag_out

            # gathered-H chunks, contiguous per partition; the leading
            # chunks are small so the first DR pairs land ~1.5us after the
            # collective while the rest stream behind the matmul chain.
            # Boundaries in k-tiles; DR pairs never straddle a chunk.
            HCH = [(0, 2), (2, 4), (4, 10), (10, 20), (20, 30), (30, 40)]
            CQ = [nc.scalar, nc.sync]

            def ag_load(layer, ag_out):
                # ag_out is [NCORES, 128, NT*BFW]; k-tile j lives in slab
                # j//NT at per-partition offset (j%NT)*BFW. Each DMA piece
                # stays inside one slab so it is contiguous per partition.
                # a tiny probe tile gates the p-state warmup: its DMA
                # issues at AG-completion and lands ~2us before chunk 0,
                # so the Tensor clock ramp starts that much earlier.
                probe = hallp.tile([128, 2, 128], fp8, tag=f"pr{layer}",
                                   name=f"probe{layer}")
                nc.scalar.dma_start(
                    out=probe[:].rearrange("p t f -> p (t f)"),
                    in_=ag_out[0][:, 0:256])
                h_q = []
                qi = 1
                for q, (s, e) in enumerate(HCH):
                    hq = hallp.tile([128, e - s, BFW], fp8,
                                    tag=f"h{layer}{q}",
                                    name=f"hall{layer}_{q}")
                    a = s
                    while a < e:
                        b = min(e, (a // NT + 1) * NT)
                        CQ[qi % 2].dma_start(
                            out=hq[:, a - s:b - s, :]
                            .rearrange("p t f -> p (t f)"),
                            in_=ag_out[a // NT]
                            [:, (a % NT) * BFW:(a % NT + b - a) * BFW])
                        qi += 1
                        a = b
                    h_q.append(hq)
                return probe, h_q

            def hsrc_slice(h_src, k2, g):
                t0 = 2 * k2
                for (s, e), tile_ in zip(HCH, h_src):
                    if s <= t0 < e:
                        return tile_[:, t0 - s:t0 - s + 2, ts(g, GW)]
                raise AssertionError(k2)

            # ---- matmul1 per group: block-diagonal grouped mm1 (fp8 DR),
            # PE-transpose back to node-major into the shared p-major
            # payload tile, then one contiguous DMA + the AllGather.
            JK2 = GB * FKT // 2      # 16 DR pairs per group
            h1sh = rowsp.tile([128, NT, BFW], fp8, tag="h1sh", name="h1sh")
            ag1_in = dramp.tile([128, NT * BFW], fp8, name="agi0")
            for g in range(NGRP):
                mp = ps.tile([128, NPAD], f32, tag="big", bufs=2,
                             name=f"mm1_{g}")
                for j2 in range(JK2):
                    nc.tensor.matmul(
                        mp[:, 0:512],
                        lhsT=wblk1_sb[:, ts(j2, 2), :],
                        rhs=xg_tiles[g][:, ts(j2, 2), 0:512],
                        perf_mode=DR,
                        start=(j2 == 0), stop=(j2 == JK2 - 1))
                    nc.tensor.matmul(
                        mp[:, 512:NPAD],
                        lhsT=wblk1_sb[:, ts(j2, 2), :],
                        rhs=xg_tiles[g][:, ts(j2, 2), 512:NPAD],
                        perf_mode=DR,
                        start=(j2 == 0), stop=(j2 == JK2 - 1))
                h1t_sb = xlp.tile([128, NPAD], bf16, tag=f"xl{g}",
                                  name=f"h1t_{g}")
                nc.scalar.copy(out=h1t_sb[:], in_=mp[:])
                trp = ps.tile([128, NT, 128], bf16, tag="mm1b", bufs=3,
                              name=f"tr_{g}")
                for t in range(NT):
                    nc.tensor.transpose(
                        trp[:, t, :], h1t_sb[:, ts(t, 128)], ident_sb[:])
                nc.scalar.copy(out=h1sh[:, :, ts(g, GW)], in_=trp[:])
            nc.sync.dma_start(
                out=ag1_in[:], in_=h1sh[:].rearrange("p t f -> p (t f)"))
            ag1 = ag_issue(0, ag1_in)

            # ---- P^T loads: deferred so X has full HBM bandwidth during mm1
            for q in range(4):
                nc.sync.dma_start(
                    out=pt_sb[:, ts(q, KT // 4), :]
                    .rearrange("p t d -> p (t d)"),
                    in_=pt_e[:, q * (KT // 4) * NPAD:
                             (q + 1) * (KT // 4) * NPAD])

            # ---- column MLP during the collective-init window (independent
            # of the GCN); produces cl_sb = col_logits + fc_b + cb2, then
            # partition-replicated per group for the DVE joint head.
            colp = ps.tile([HF, B * C], f32, tag="big", bufs=2, name="colp")
            for h in range(2):
                nc.tensor.matmul(colp[:, ts(h, 512)], lhsT=cw1_sb[:],
                                 rhs=cft_sb[:, ts(h, 512)],
                                 start=True, stop=True)
            hcol_sb = constp.tile([HF, B * C], bf16, name="hcol_sb")
            nc.scalar.activation(out=hcol_sb[:], in_=colp[:], func=AF.Relu,
                                 bias=cb1_sb[:, 0:1])
            clp = ps.tile([1, B * C], f32, tag="big", bufs=2, name="clp")
            for h in range(2):
                nc.tensor.matmul(clp[:, ts(h, 512)], lhsT=cw2_sb[:],
                                 rhs=hcol_sb[:, ts(h, 512)],
                                 start=True, stop=True)
            cl_sb = constp.tile([1, B * C], bf16, name="cl_sb")
            nc.scalar.activation(out=cl_sb[:], in_=clp[:], func=AF.Identity,
                                 bias=clb_sb[:, 0:1])
            clrep_sb = []
            for g in range(NGRP):
                crp = ps.tile([128, GB * C], f32, tag="mm1b", bufs=3,
                              name=f"crp_{g}")
                nc.tensor.matmul(crp[:], lhsT=ones_sb[:],
                                 rhs=cl_sb[0:1, ts(g, GB * C)],
                                 start=True, stop=True)
                crs = constp.tile([128, GB, C], bf16, name=f"clrep_{g}")
                nc.scalar.copy(
                    out=crs[:],
                    in_=crp[:].rearrange("p (b c) -> p b c", b=GB))
                clrep_sb.append(crs)

            def agg(ap_, h_src, g, warm=0, probe=None):
                # two independent accumulation chains: cols 0:512 (A) and
                # 512:640 (B). `warm` scratch 128-wide matmuls gated on
                # chunk 0 lead the chain: they start exactly when the
                # first gathered data lands (no static timing), ramp the
                # Tensor clock out of its idle p-state on cheap work, and
                # let the chunk stream build headroom before the real
                # chains consume it.
                def mmA(k2):
                    nc.tensor.matmul(
                        ap_[:, 0:512], lhsT=hsrc_slice(h_src, k2, g),
                        rhs=pt_sb[:, ts(k2, 2), 0:512],
                        perf_mode=DR,
                        start=(k2 == 0), stop=(k2 == KT2 - 1))

                def mmB(k2):
                    nc.tensor.matmul(
                        ap_[:, 512:NPAD], lhsT=hsrc_slice(h_src, k2, g),
                        rhs=pt_sb[:, ts(k2, 2), 512:NPAD],
                        perf_mode=DR,
                        start=(k2 == 0), stop=(k2 == KT2 - 1))

                # high-duty warm ops (512-wide DR streams ~56% duty vs
                # ~17% for 128-wide) ramp the idle p-state much faster
                for _ in range(warm):
                    nc.tensor.matmul(
                        warm_ps[:], lhsT=probe[:],
                        rhs=pt_sb[:, 0:2, 0:512],
                        perf_mode=DR, start=True, stop=True)
                for k2 in range(KT2):
                    mmA(k2)
                    mmB(k2)

            # p-state warmup: independent matmuls into a scratch psum keep
            # the Tensor clock at 2.4GHz through the AllGather waits (a
            # cold chain runs at ~1.2GHz for its first ~5us).
            warm_ps = ps.tile([128, 512], f32, tag="warm", bufs=1,
                              name="warm_ps")

            def fillers(n):
                # K=1 bf16 ops: high stream duty (keeps the DVFS clock up)
                # at ~1% of the MAC power of fp8-DR fillers, so they do
                # not heat-throttle the in-flight collective.
                for _ in range(n):
                    nc.tensor.matmul(
                        warm_ps[:], lhsT=ones_sb[:],
                        rhs=cl_sb[0:1, 0:512],
                        start=True, stop=True)

            def relu_piece(x_g, ap_, bias_sb, piece):
                if piece == 0:
                    nc.scalar.activation(out=x_g[:, 0:256], in_=ap_[:, 0:256],
                                         func=AF.Relu, bias=bias_sb[:, 0:1])
                    nc.vector.tensor_scalar(
                        out=x_g[:, 256:512], in0=ap_[:, 256:512],
                        scalar1=bias_sb[:, 0:1], scalar2=0.0,
                        op0=mybir.AluOpType.add, op1=mybir.AluOpType.max)
                else:
                    nc.vector.tensor_scalar(
                        out=x_g[:, 512:NPAD], in0=ap_[:, 512:NPAD],
                        scalar1=bias_sb[:, 0:1], scalar2=0.0,
                        op0=mybir.AluOpType.add, op1=mybir.AluOpType.max)

            # ---- layer 1: aggregate per group with interleaved, piecewise
            # epilogues (relu/mm2/copies of tiles 0-3 run under the 128-col
            # chain; group 0's epilogue runs under group 1's chain).
            fillers(N_FILL1)
            pr1, h1_src = ag_load(0, ag1)
            h2sh = rowsp.tile([128, NT, BFW], fp8, tag="h2sh", name="h2sh")
            ag2_in = dramp.tile([128, NT, BFW], fp8, name="agi1")
            for g in range(NGRP):
                ap_ = ps.tile([128, NPAD], f32, tag="big", bufs=2,
                              name=f"agg0_{g}")
                agg(ap_, h1_src, g, warm=6 if g == 0 else 0, probe=pr1)
                x_g = xlp.tile([128, NPAD], bf16, tag=f"xl{g}",
                               name=f"xl0_{g}")
                mp2 = ps.tile([128, NPAD], f32, tag="big", bufs=2,
                              name=f"mm2_{g}")
                relu_piece(x_g, ap_, b1_sb, 0)
                for t in range(4):
                    nc.tensor.matmul(mp2[:, ts(t, 128)],
                                     lhsT=x_g[:, ts(t, 128)],
                                     rhs=wblk_sb[:],
                                     start=True, stop=True)
                nc.scalar.copy(
                    out=h2sh[:, 0:2, ts(g, GW)],
                    in_=mp2[:, 0:256].rearrange("p (t f) -> p t f", t=2))
                nc.vector.tensor_copy(
                    h2sh[:, 2:4, ts(g, GW)],
                    mp2[:, 256:512].rearrange("p (t f) -> p t f", t=2))
                if g == 1:
                    # full-width contiguous payload DMA (group 0's columns
                    # are long since written): ~5x faster than per-group
                    # 128B-strided slices and one less issue on the
                    # AG2-trigger path.
                    nc.sync.dma_start(
                        out=ag2_in[:, 0:4, :].rearrange("p t f -> p (t f)"),
                        in_=h2sh[:, 0:4, :].rearrange("p t f -> p (t f)"))
                relu_piece(x_g, ap_, b1_sb, 1)
                nc.tensor.matmul(mp2[:, 512:NPAD],
                                 lhsT=x_g[:, 512:NPAD],
                                 rhs=wblk_sb[:], start=True, stop=True)
                nc.scalar.copy(
                    out=h2sh[:, 4, ts(g, GW)], in_=mp2[:, 512:NPAD])
                if g == 1:
                    nc.scalar.dma_start(out=ag2_in[:, 4, :],
                                        in_=h2sh[:, 4, :])
            ag2 = ag_issue(1, ag2_in)

            # ---- layer 2 + fused joint head per group: narrow fc matmul
            # nl[dst, b] then one DVE broadcast-add per node tile:
            # st2[dst, (b,c)] = nl[dst, b] + clrep[dst, (b,c)].
            fillers(N_FILL2)
            pr2, h2_src = ag_load(1, ag2)
            for g in range(NGRP):
                ap_ = ps.tile([128, NPAD], f32, tag="big", bufs=2,
                              name=f"agg1_{g}")
                agg(ap_, h2_src, g, warm=6 if g == 0 else 0, probe=pr2)
                x_g = xlp.tile([128, NPAD], bf16, tag=f"xl{g}",
                               name=f"xl2_{g}")
                st2 = stg2p.tile([128, NT, GB * C], bf16, tag=f"st{g}",
                                 name=f"st2_{g}")
                nlp = ps.tile([128, NT, GB, 1], f32, tag="mm1b", bufs=3,
                              name=f"nl_{g}")
                relu_piece(x_g, ap_, b2_sb, 0)
                for t in (0, 2):
                    nc.tensor.matmul(
                        nlp[:, t, :, 0], lhsT=x_g[:, ts(t, 128)],
                        rhs=fcblk_sb[:], start=True, stop=True)
                jts = []
                for t in (1, 3):
                    jt = ps.tile([128, GB * C], f32, tag="mm1b", bufs=3,
                                 name=f"jt_{g}_{t}")
                    nc.tensor.matmul(jt[:], lhsT=x_g[:, ts(t, 128)],
                                     rhs=fcbc_sb[:], start=True, stop=False)
                    nc.tensor.matmul(jt[:], lhsT=ones_sb[:],
                                     rhs=cl_sb[0:1, ts(g, GB * C)],
                                     start=False, stop=True)
                    jts.append(jt)
                relu_piece(x_g, ap_, b2_sb, 1)
                nc.tensor.matmul(
                    nlp[:, 4, :, 0], lhsT=x_g[:, 512:NPAD],
                    rhs=fcblk_sb[:], start=True, stop=True)

                def bcast(t):
                    nl_b, cr_b = broadcast_tensor_aps(
                        nlp[:, t, :, :], clrep_sb[g][:])
                    nc.vector.scalar_tensor_tensor(
                        out=st2[:, t, :].rearrange("p (b c) -> p b c", b=GB),
                        in0=nl_b, scalar=0.0, in1=cr_b,
                        op0=mybir.AluOpType.add, op1=mybir.AluOpType.add)

                # even tiles on the DVE (broadcast-add), odd tiles on the
                # idle Tensor engine (fcbc matmul) + scalar copies.
                bcast(0)
                nc.scalar.copy(out=st2[:, 1, :], in_=jts[0][:])
                nc.sync.dma_start(
                    out=out_e[g][:, 0:2 * GB * C],
                    in_=st2[:, 0:2, :].rearrange("p t bc -> p (t bc)"))
                bcast(2)
                nc.scalar.copy(out=st2[:, 3, :], in_=jts[1][:])
                nc.scalar.dma_start(
                    out=out_e[g][:, 2 * GB * C:4 * GB * C],
                    in_=st2[:, 2:4, :].rearrange("p t bc -> p (t bc)"))
                bcast(4)
                nc.sync.dma_start(
                    out=out_e[g][:, 4 * GB * C:],
                    in_=st2[:, 4, :])

    nc.compile()
    return nc


def _get_graph():
    if "nc" not in _GRAPH_CACHE:
        _GRAPH_CACHE["nc"] = _build_graph()
    return _GRAPH_CACHE["nc"]


# --------------------------------------------------------------------------
# Entry point
# --------------------------------------------------------------------------

def _run(inputs, trace=False, tmpdir=None):
    from concourse.bass_utils import run_bass_kernel_spmd

    xt_cores, pt_cores, shared = _preprocess(inputs)
    nc = _get_graph()
    in_maps = []
    for c in range(NCORES):
        m = dict(shared)
        m["xt"] = xt_cores[c]
        m["pt"] = pt_cores[c]
        in_maps.append(m)
    res = run_bass_kernel_spmd(nc, in_maps, core_ids=list(range(NCORES)),
                               trace=trace, tmpdir=tmpdir)
    out = np.zeros((B, N, C), dtype=np.float32)
    for c in range(NCORES):
        o = np.asarray(res.results[c]["out"]).astype(np.float32)
        o = (o.reshape(NGRP, 128, NT, GB, C).transpose(0, 3, 2, 1, 4)
             .reshape(B, NPAD, C))
        out[:, c * NLOC:(c + 1) * NLOC, :] = o[:, :NLOC, :]
    return out.reshape(B, N * C), res


def kernel(**inputs) -> np.ndarray:
    out, _ = _run(inputs, trace=False)
    return out
